# revision 29
# baseline (speedup 1.0000x reference)
"""MoE block kernel for 8 TRN2 NeuronCores (expert-parallel).

Strategy:
  - Router (tiny [T,H]@[H,E] matmul + top-2 + softmax) on host; dispatch is
    folded into input sharding: each core receives the tokens routed to its
    expert (gathered, zero-padded to CAP, transposed to [H, CAP]).
  - Core i holds expert i's weights and computes the expert FFN densely on
    its gathered tokens. Activations stay in SBUF end-to-end (x, silu(g),
    h=silu(g)*u are SBUF-resident caches); each weight byte is read from
    HBM exactly once, from a host-pre-tiled contiguous block layout so DMA
    runs are 4KB. Matmuls run in float32r (fp32 storage, full PE rate for
    moving dim >= 256).
  - Shared expert is token-sharded: core i computes the full shared FFN for
    tokens [256*i, 256*(i+1)) (weights replicated).
  - Device outputs are feature-major ([H, n]); host unshard: concat shared
    slices, scatter-add combine-weighted expert outputs.
"""
import sys, threading
sys.path.insert(0, "/opt/trn_rl_repo")
import numpy as np

H = 1024
F = 1024
E = 8
K = 2
T = 2048
CAP = 768          # per-expert token capacity (max observed load 551)
TSH = T // 8       # shared-expert token slice per core

P = 128
KT = 4             # K_SUBTILES (K_TILE=512)
MT = 256           # M_TILE for weight blocks
KO = H // (P * KT)          # 2 k-tiles
MB = F // MT                # 4 m-tiles
NBLK = KO * MB              # 8 blocks per weight matrix

USE_BF16 = False   # False: float32r matmuls (fp32 storage, ~3e-4 rel err,
                   # ~136us). True: bf16 weights/acts (~4e-3 rel err, ~114us).

_cache = {}
_lock = threading.Lock()


def _build_nc():
    import concourse.mybir as mybir
    import concourse.tile as tile
    from concourse import bacc
    from concourse.bass import ds, ts
    from concourse.kernels.tile_matmul import (
        composable_matmul_tile_kernel, dma_to_dram_mxn, scalar_copyback,
        ShapeInfo, TileKxM)

    dt = mybir.dt.bfloat16 if USE_BF16 else mybir.dt.float32r
    dt_out = mybir.dt.float32

    nc = bacc.Bacc("TRN2", target_bir_lowering=False, debug=False, num_devices=8)

    # x inputs pre-tiled to [P, (H/P)*n] (partition-inner layout)
    xg = nc.dram_tensor("xg", [P, (H // P) * CAP], dt, kind="ExternalInput")
    xs = nc.dram_tensor("xs", [P, (H // P) * TSH], dt, kind="ExternalInput")
    # weights pre-tiled to [KO, P, KT*F]: one contiguous 2MB block per K-half
    wnames = ["wg", "wu", "wd", "wsg", "wsu", "wsd"]
    wt = {n: nc.dram_tensor(n, [KO, P, KT * F], dt, kind="ExternalInput")
          for n in wnames}
    # feature-major outputs ([H, tokens]); host transposes
    yexp = nc.dram_tensor("yexp", [H, CAP], dt_out, kind="ExternalOutput")
    shs = nc.dram_tensor("shs", [H, TSH], dt_out, kind="ExternalOutput")

    w_shape = ShapeInfo(pdims=((P, H // P),), fdims=(F,))

    with tile.TileContext(nc) as tc:
        with tc.tile_pool(name="wpool", bufs=4) as wpool, \
             tc.tile_pool(name="acts", bufs=1) as acts:

            def w_producer(dram_t):
                # one 2MB DMA per K-half of each matrix; producer slices the
                # SBUF-resident half per m_tile
                halves = {}

                def prod(nc_, md):
                    assert md.k_subtiles == KT and md.m_tile == MT
                    kt = md.k_tile_idx
                    if kt not in halves:
                        t = wpool.tile([P, KT, F], dt, tag="whalf")
                        nc_.sync.dma_start(
                            t[:],
                            dram_t.ap()[kt].rearrange(
                                "p (ks m) -> p ks m", ks=KT))
                        halves[kt] = t
                    return halves[kt][:, :, ts(md.m_tile_idx, MT)]
                return prod

            def load_cache(dram_t, n, name, n_tile):
                # One tile per (k_tile, n_tile) chunk so the first matmuls
                # only depend on the chunk they actually read.
                v = dram_t.ap().rearrange("p (po f) -> p po f", po=H // P)
                tiles = {}
                for kt in range(H // (P * KT)):
                    for nt in range(n // n_tile):
                        t = acts.tile([P, KT, n_tile], dt,
                                      name=f"{name}_{kt}_{nt}")
                        nc.sync.dma_start(
                            t[:], v[:, ts(kt, KT), ds(nt * n_tile, n_tile)])
                        tiles[(kt, nt)] = t
                return tiles

            def cached_kxn(tiles, k_chunks, n_tok):
                def prod(nc_, md):
                    t = tiles.get((md.k_tile_idx, md.n_tile_idx))
                    if t is not None:
                        assert t.shape[1] == md.k_subtiles
                        return t[:]
                    # fall back for single-tile caches (h/sg)
                    raise KeyError((md.k_tile_idx, md.n_tile_idx))
                return prod, ShapeInfo(pdims=((P, k_chunks),), fdims=(n_tok,))

            def cached_kxn_single(cache_tile, k_chunks, n_tok):
                def prod(nc_, md):
                    return cache_tile[
                        :, ts(md.k_tile_idx, md.k_subtiles),
                        ds(md.n_tile_idx * md.n_tile, md.n_tile)]
                return prod, ShapeInfo(pdims=((P, k_chunks),), fdims=(n_tok,))

            def prewarm(prod, kt):
                prod(nc, TileKxM(
                    k_batch_idx=0, k_tile_idx=kt, k_tile=P * KT, k_subtiles=KT,
                    k_subtile=P, m_batch_idx=0, m_tile_idx=0, m_tile=MT,
                    m_subtiles=MT // P, m_subtile=P))

            def ffn_fused(x_sb, n_tok, w1_prod, w2_prod, w3_prod, outT, tag,
                          max_tile):
                sg = acts.tile([P, F // P, n_tok], dt, name=f"sg_{tag}")
                h = acts.tile([P, F // P, n_tok], dt, name=f"h_{tag}")

                x_prod, x_shape = cached_kxn(x_sb, H // P, n_tok)
                h_prod, h_shape = cached_kxn_single(h, F // P, n_tok)

                def prod_sg(nc_, md):
                    return sg[:, ts(md.m_tile_idx, md.m_subtiles),
                              ds(md.n_tile_idx * md.n_tile, md.n_tile)]

                def prod_h(nc_, md):
                    return h[:, ts(md.m_tile_idx, md.m_subtiles),
                             ds(md.n_tile_idx * md.n_tile, md.n_tile)]

                def red_silu(nc_, psum, sbuf, md):
                    nc_.scalar.activation(
                        sbuf[:], psum[:, :md.n_subtile_slice_size],
                        mybir.ActivationFunctionType.Silu)

                def red_mul(nc_, psum, sbuf, md):
                    po = md.m_tile_idx * md.m_subtiles + md.m_subtile_idx
                    n_lo = md.n_tile_idx * md.n_tile + md.n_subtile_idx * md.n_subtile
                    nsz = md.n_subtile_slice_size
                    nc_.vector.tensor_mul(
                        out=sbuf[:], in0=psum[:, :nsz],
                        in1=sg[:, po:po + 1, ds(n_lo, nsz)])

                def noop_consumer(nc_, sbuf, md):
                    pass

                common = dict(MAX_TILE_SIZE=max_tile, MAX_K_TILE_SIZE=P * KT,
                              cache_tiles=True, psum_n_bufs=2)

                # sg = silu(w1.T @ x)
                composable_matmul_tile_kernel(
                    tc, kxm_shape=w_shape, kxn_shape=x_shape, output_type=None,
                    kxm_producer=w1_prod, kxn_producer=x_prod,
                    mxn_consumer=noop_consumer, mxn_subtile_reducer=red_silu,
                    mxn_subtile_producer=prod_sg, **common)
                # h = sg * (w2.T @ x)
                composable_matmul_tile_kernel(
                    tc, kxm_shape=w_shape, kxn_shape=x_shape, output_type=None,
                    kxm_producer=w2_prod, kxn_producer=x_prod,
                    mxn_consumer=noop_consumer, mxn_subtile_reducer=red_mul,
                    mxn_subtile_producer=prod_h, **common)
                # outT = w3.T @ h   ([H, n_tok])
                composable_matmul_tile_kernel(
                    tc, kxm_shape=w_shape, kxn_shape=h_shape, output_type=dt_out,
                    kxm_producer=w3_prod, kxn_producer=h_prod,
                    mxn_consumer=dma_to_dram_mxn(outT.ap()),
                    mxn_subtile_reducer=scalar_copyback(), **common)

            wprods = {n: w_producer(wt[n]) for n in wnames}
            # first expert-weight halves load ahead of the x caches so the
            # first matmuls' inputs stream concurrently
            prewarm(wprods["wg"], 0)
            xg_sb = load_cache(xg, CAP, "xg_sb", 384)
            prewarm(wprods["wg"], 1)
            prewarm(wprods["wu"], 0)
            xs_sb = load_cache(xs, TSH, "xs_sb", 256)
            ffn_fused(xg_sb, CAP, wprods["wg"], wprods["wu"], wprods["wd"],
                      yexp, "e", 384)
            ffn_fused(xs_sb, TSH, wprods["wsg"], wprods["wsu"], wprods["wsd"],
                      shs, "s", 256)

    nc.compile()
    return nc


def _get_nc():
    with _lock:
        if "nc" not in _cache:
            _cache["nc"] = _build_nc()
        return _cache["nc"]


def _dev_dtype():
    if USE_BF16:
        import ml_dtypes
        return ml_dtypes.bfloat16
    return np.float32


def _tile_w(w):
    """[K=1024, M=1024] -> [KO, P, KT*M]: per K-half, per-partition
    contiguous runs laid out (ks, m).  k = ko*512 + ks*128 + pi.
    """
    return np.ascontiguousarray(
        w.reshape(KO, KT, P, F).transpose(0, 2, 1, 3).reshape(KO, P, KT * F)
         .astype(_dev_dtype()))


def _tile_x(xT_cols):
    """[H=1024, n] -> [P, (H/P)*n] partition-inner layout."""
    n = xT_cols.shape[1]
    return np.ascontiguousarray(
        xT_cols.reshape(H // P, P, n).transpose(1, 0, 2)
               .reshape(P, (H // P) * n).astype(_dev_dtype()))


def _route(x, w_gate):
    """Host router: top-2 + softmax, replicating the reference's math."""
    logits = x.astype(np.float64) @ w_gate.astype(np.float64)   # [T, E]
    order = np.argsort(-logits, axis=-1, kind="stable")
    idx = order[:, :K]                                          # [T, K]
    vals = np.take_along_axis(logits, idx, axis=-1).astype(np.float32)
    m = vals.max(axis=-1, keepdims=True)
    ex = np.exp(vals - m)
    wts = (ex / ex.sum(axis=-1, keepdims=True)).astype(np.float32)  # [T, K]
    return idx, wts


def _np_expert(x_tok, wg, wu, wd):
    """Host fp32 fallback for capacity-overflow tokens."""
    g = x_tok @ wg
    u = x_tok @ wu
    h = (g * (1.0 / (1.0 + np.exp(-g)))) * u
    return h @ wd


def kernel(hidden_states, w_gate, w_gate_proj, w_up_proj, w_down_proj,
           w_shared_gate, w_shared_up, w_shared_down):
    from concourse.bass_utils import run_bass_kernel_spmd

    x = np.ascontiguousarray(np.asarray(hidden_states, dtype=np.float32)
                             .reshape(T, H))
    w_gate = np.asarray(w_gate, dtype=np.float32)
    wgp = np.asarray(w_gate_proj, dtype=np.float32)
    wup = np.asarray(w_up_proj, dtype=np.float32)
    wdp = np.asarray(w_down_proj, dtype=np.float32)
    wsg_t = _tile_w(np.asarray(w_shared_gate, dtype=np.float32))
    wsu_t = _tile_w(np.asarray(w_shared_up, dtype=np.float32))
    wsd_t = _tile_w(np.asarray(w_shared_down, dtype=np.float32))

    idx, wts = _route(x, w_gate)

    xT = np.ascontiguousarray(x.T)  # [H, T]

    tok_ids = []    # per expert: token ids routed to it (device portion)
    tok_w = []      # per expert: combine weights for those tokens
    over_ids = []   # per expert: overflow token ids (host fallback)
    over_w = []
    in_maps = []
    for e in range(E):
        sel = np.nonzero(idx == e)       # (token, slot) pairs
        t_ids = sel[0]
        t_w = wts[sel[0], sel[1]]
        dev_ids, dev_w = t_ids[:CAP], t_w[:CAP]
        over_ids.append(t_ids[CAP:])
        over_w.append(t_w[CAP:])
        tok_ids.append(dev_ids)
        tok_w.append(dev_w)

        xgT = np.zeros((H, CAP), dtype=np.float32)
        xgT[:, :len(dev_ids)] = xT[:, dev_ids]
        in_maps.append({
            "xg": _tile_x(xgT),
            "xs": _tile_x(xT[:, e * TSH:(e + 1) * TSH]),
            "wg": _tile_w(wgp[e]),
            "wu": _tile_w(wup[e]),
            "wd": _tile_w(wdp[e]),
            "wsg": wsg_t,
            "wsu": wsu_t,
            "wsd": wsd_t,
        })

    nc = _get_nc()
    res = run_bass_kernel_spmd(nc, in_maps, core_ids=list(range(8))).results

    out = np.empty((T, H), dtype=np.float32)
    for c in range(8):
        out[c * TSH:(c + 1) * TSH] = res[c]["shs"].T
    for e in range(E):
        n = len(tok_ids[e])
        if n:
            out[tok_ids[e]] += (res[e]["yexp"][:, :n] * tok_w[e][None, :]).T
        if len(over_ids[e]):
            y = _np_expert(x[over_ids[e]], wgp[e], wup[e], wdp[e])
            out[over_ids[e]] += over_w[e][:, None] * y

    return out.reshape(1, T, H), np.float32(0.0)


# revision 31
# speedup vs baseline: 1.2283x; 1.2283x over previous
"""MoE block kernel for 8 TRN2 NeuronCores (expert-parallel).

Strategy:
  - Router (tiny [T,H]@[H,E] matmul + top-2 + softmax) on host; dispatch is
    folded into input sharding: each core receives the tokens routed to its
    expert (gathered, zero-padded to CAP, transposed to [H, CAP]).
  - Core i holds expert i's weights and computes the expert FFN densely on
    its gathered tokens. Activations stay in SBUF end-to-end (x, silu(g),
    h=silu(g)*u are SBUF-resident caches); each weight byte is read from
    HBM exactly once, from a host-pre-tiled contiguous block layout so DMA
    runs are 4KB. Matmuls run in float32r (fp32 storage, full PE rate for
    moving dim >= 256).
  - Shared expert is token-sharded: core i computes the full shared FFN for
    tokens [256*i, 256*(i+1)) (weights replicated).
  - Device outputs are feature-major ([H, n]); host unshard: concat shared
    slices, scatter-add combine-weighted expert outputs.
"""
import sys, threading
sys.path.insert(0, "/opt/trn_rl_repo")
import numpy as np

H = 1024
F = 1024
E = 8
K = 2
T = 2048
CAP = 512          # device capacity; overflow tokens -> exact host fallback
TSH = T // 8       # shared-expert token slice per core

P = 128
KT = 4             # K_SUBTILES (K_TILE=512)
MT = 256           # M_TILE for weight blocks
KO = H // (P * KT)          # 2 k-tiles
MB = F // MT                # 4 m-tiles
NBLK = KO * MB              # 8 blocks per weight matrix

USE_BF16 = False   # False: float32r matmuls (fp32 storage, ~3e-4 rel err,
                   # ~136us). True: bf16 weights/acts (~4e-3 rel err, ~114us).

_cache = {}
_lock = threading.Lock()


def _build_nc():
    import concourse.mybir as mybir
    import concourse.tile as tile
    from concourse import bacc
    from concourse.bass import ds, ts
    from concourse.kernels.tile_matmul import (
        composable_matmul_tile_kernel, dma_to_dram_mxn, scalar_copyback,
        ShapeInfo, TileKxM)

    dt = mybir.dt.bfloat16 if USE_BF16 else mybir.dt.float32r
    dt_out = mybir.dt.float32

    nc = bacc.Bacc("TRN2", target_bir_lowering=False, debug=False, num_devices=8)

    # x inputs pre-tiled to [P, (H/P)*n] (partition-inner layout)
    xg = nc.dram_tensor("xg", [P, (H // P) * CAP], dt, kind="ExternalInput")
    xs = nc.dram_tensor("xs", [P, (H // P) * TSH], dt, kind="ExternalInput")
    # weights pre-tiled to [KO, P, KT*F]: one contiguous 2MB block per K-half
    wnames = ["wg", "wu", "wd", "wsg", "wsu", "wsd"]
    wt = {n: nc.dram_tensor(n, [KO, P, KT * F], dt, kind="ExternalInput")
          for n in wnames}
    # feature-major outputs ([H, tokens]); host transposes
    yexp = nc.dram_tensor("yexp", [H, CAP], dt_out, kind="ExternalOutput")
    shs = nc.dram_tensor("shs", [H, TSH], dt_out, kind="ExternalOutput")

    w_shape = ShapeInfo(pdims=((P, H // P),), fdims=(F,))

    with tile.TileContext(nc) as tc:
        with tc.tile_pool(name="wpool", bufs=4) as wpool, \
             tc.tile_pool(name="acts", bufs=1) as acts:

            def w_producer(dram_t):
                # one 2MB DMA per K-half of each matrix; producer slices the
                # SBUF-resident half per m_tile
                halves = {}

                def prod(nc_, md):
                    assert md.k_subtiles == KT
                    kt = md.k_tile_idx
                    if kt not in halves:
                        t = wpool.tile([P, KT, F], dt, tag="whalf")
                        nc_.sync.dma_start(
                            t[:],
                            dram_t.ap()[kt].rearrange(
                                "p (ks m) -> p ks m", ks=KT))
                        halves[kt] = t
                    return halves[kt][:, :, ts(md.m_tile_idx, md.m_tile)]
                return prod

            def load_cache(dram_t, n, name, n_tile):
                # One tile per (k_tile, n_tile) chunk so the first matmuls
                # only depend on the chunk they actually read.
                v = dram_t.ap().rearrange("p (po f) -> p po f", po=H // P)
                tiles = {}
                for kt in range(H // (P * KT)):
                    for nt in range(n // n_tile):
                        t = acts.tile([P, KT, n_tile], dt,
                                      name=f"{name}_{kt}_{nt}")
                        nc.sync.dma_start(
                            t[:], v[:, ts(kt, KT), ds(nt * n_tile, n_tile)])
                        tiles[(kt, nt)] = t
                return tiles

            def cached_kxn(tiles, k_chunks, n_tok):
                def prod(nc_, md):
                    t = tiles.get((md.k_tile_idx, md.n_tile_idx))
                    if t is not None:
                        assert t.shape[1] == md.k_subtiles
                        return t[:]
                    # fall back for single-tile caches (h/sg)
                    raise KeyError((md.k_tile_idx, md.n_tile_idx))
                return prod, ShapeInfo(pdims=((P, k_chunks),), fdims=(n_tok,))

            def cached_kxn_single(cache_tile, k_chunks, n_tok):
                def prod(nc_, md):
                    return cache_tile[
                        :, ts(md.k_tile_idx, md.k_subtiles),
                        ds(md.n_tile_idx * md.n_tile, md.n_tile)]
                return prod, ShapeInfo(pdims=((P, k_chunks),), fdims=(n_tok,))

            def prewarm(prod, kt):
                prod(nc, TileKxM(
                    k_batch_idx=0, k_tile_idx=kt, k_tile=P * KT, k_subtiles=KT,
                    k_subtile=P, m_batch_idx=0, m_tile_idx=0, m_tile=MT,
                    m_subtiles=MT // P, m_subtile=P))

            def ffn_fused(x_sb, n_tok, w1_prod, w2_prod, w3_prod, outT, tag,
                          max_tile):
                sg = acts.tile([P, F // P, n_tok], dt, name=f"sg_{tag}")
                h = acts.tile([P, F // P, n_tok], dt, name=f"h_{tag}")

                x_prod, x_shape = cached_kxn(x_sb, H // P, n_tok)
                h_prod, h_shape = cached_kxn_single(h, F // P, n_tok)

                def prod_sg(nc_, md):
                    return sg[:, ts(md.m_tile_idx, md.m_subtiles),
                              ds(md.n_tile_idx * md.n_tile, md.n_tile)]

                def prod_h(nc_, md):
                    return h[:, ts(md.m_tile_idx, md.m_subtiles),
                             ds(md.n_tile_idx * md.n_tile, md.n_tile)]

                def red_silu(nc_, psum, sbuf, md):
                    nc_.scalar.activation(
                        sbuf[:], psum[:, :md.n_subtile_slice_size],
                        mybir.ActivationFunctionType.Silu)

                def red_mul(nc_, psum, sbuf, md):
                    po = md.m_tile_idx * md.m_subtiles + md.m_subtile_idx
                    n_lo = md.n_tile_idx * md.n_tile + md.n_subtile_idx * md.n_subtile
                    nsz = md.n_subtile_slice_size
                    nc_.vector.tensor_mul(
                        out=sbuf[:], in0=psum[:, :nsz],
                        in1=sg[:, po:po + 1, ds(n_lo, nsz)])

                def noop_consumer(nc_, sbuf, md):
                    pass

                common = dict(MAX_TILE_SIZE=max_tile, MAX_K_TILE_SIZE=P * KT,
                              cache_tiles=True, psum_n_bufs=2)

                # sg = silu(w1.T @ x)
                composable_matmul_tile_kernel(
                    tc, kxm_shape=w_shape, kxn_shape=x_shape, output_type=None,
                    kxm_producer=w1_prod, kxn_producer=x_prod,
                    mxn_consumer=noop_consumer, mxn_subtile_reducer=red_silu,
                    mxn_subtile_producer=prod_sg, **common)
                # h = sg * (w2.T @ x)
                composable_matmul_tile_kernel(
                    tc, kxm_shape=w_shape, kxn_shape=x_shape, output_type=None,
                    kxm_producer=w2_prod, kxn_producer=x_prod,
                    mxn_consumer=noop_consumer, mxn_subtile_reducer=red_mul,
                    mxn_subtile_producer=prod_h, **common)
                # outT = w3.T @ h   ([H, n_tok])
                composable_matmul_tile_kernel(
                    tc, kxm_shape=w_shape, kxn_shape=h_shape, output_type=dt_out,
                    kxm_producer=w3_prod, kxn_producer=h_prod,
                    mxn_consumer=dma_to_dram_mxn(outT.ap()),
                    mxn_subtile_reducer=scalar_copyback(), **common)

            wprods = {n: w_producer(wt[n]) for n in wnames}
            # first expert-weight halves load ahead of the x caches so the
            # first matmuls' inputs stream concurrently
            prewarm(wprods["wg"], 0)
            xg_sb = load_cache(xg, CAP, "xg_sb", 512)
            prewarm(wprods["wg"], 1)
            prewarm(wprods["wu"], 0)
            xs_sb = load_cache(xs, TSH, "xs_sb", 256)
            ffn_fused(xg_sb, CAP, wprods["wg"], wprods["wu"], wprods["wd"],
                      yexp, "e", 512)
            ffn_fused(xs_sb, TSH, wprods["wsg"], wprods["wsu"], wprods["wsd"],
                      shs, "s", 256)

    nc.compile()
    return nc


def _get_nc():
    with _lock:
        if "nc" not in _cache:
            _cache["nc"] = _build_nc()
        return _cache["nc"]


def _dev_dtype():
    if USE_BF16:
        import ml_dtypes
        return ml_dtypes.bfloat16
    return np.float32


def _tile_w(w):
    """[K=1024, M=1024] -> [KO, P, KT*M]: per K-half, per-partition
    contiguous runs laid out (ks, m).  k = ko*512 + ks*128 + pi.
    """
    return np.ascontiguousarray(
        w.reshape(KO, KT, P, F).transpose(0, 2, 1, 3).reshape(KO, P, KT * F)
         .astype(_dev_dtype()))


def _tile_x(xT_cols):
    """[H=1024, n] -> [P, (H/P)*n] partition-inner layout."""
    n = xT_cols.shape[1]
    return np.ascontiguousarray(
        xT_cols.reshape(H // P, P, n).transpose(1, 0, 2)
               .reshape(P, (H // P) * n).astype(_dev_dtype()))


def _route(x, w_gate):
    """Host router: top-2 + softmax, replicating the reference's math."""
    logits = x.astype(np.float64) @ w_gate.astype(np.float64)   # [T, E]
    order = np.argsort(-logits, axis=-1, kind="stable")
    idx = order[:, :K]                                          # [T, K]
    vals = np.take_along_axis(logits, idx, axis=-1).astype(np.float32)
    m = vals.max(axis=-1, keepdims=True)
    ex = np.exp(vals - m)
    wts = (ex / ex.sum(axis=-1, keepdims=True)).astype(np.float32)  # [T, K]
    return idx, wts


def _np_expert(x_tok, wg, wu, wd):
    """Host fp32 fallback for capacity-overflow tokens."""
    g = x_tok @ wg
    u = x_tok @ wu
    h = (g * (1.0 / (1.0 + np.exp(-g)))) * u
    return h @ wd


def kernel(hidden_states, w_gate, w_gate_proj, w_up_proj, w_down_proj,
           w_shared_gate, w_shared_up, w_shared_down):
    from concourse.bass_utils import run_bass_kernel_spmd

    x = np.ascontiguousarray(np.asarray(hidden_states, dtype=np.float32)
                             .reshape(T, H))
    w_gate = np.asarray(w_gate, dtype=np.float32)
    wgp = np.asarray(w_gate_proj, dtype=np.float32)
    wup = np.asarray(w_up_proj, dtype=np.float32)
    wdp = np.asarray(w_down_proj, dtype=np.float32)
    wsg_t = _tile_w(np.asarray(w_shared_gate, dtype=np.float32))
    wsu_t = _tile_w(np.asarray(w_shared_up, dtype=np.float32))
    wsd_t = _tile_w(np.asarray(w_shared_down, dtype=np.float32))

    idx, wts = _route(x, w_gate)

    xT = np.ascontiguousarray(x.T)  # [H, T]

    tok_ids = []    # per expert: token ids routed to it (device portion)
    tok_w = []      # per expert: combine weights for those tokens
    over_ids = []   # per expert: overflow token ids (host fallback)
    over_w = []
    in_maps = []
    for e in range(E):
        sel = np.nonzero(idx == e)       # (token, slot) pairs
        t_ids = sel[0]
        t_w = wts[sel[0], sel[1]]
        dev_ids, dev_w = t_ids[:CAP], t_w[:CAP]
        over_ids.append(t_ids[CAP:])
        over_w.append(t_w[CAP:])
        tok_ids.append(dev_ids)
        tok_w.append(dev_w)

        xgT = np.zeros((H, CAP), dtype=np.float32)
        xgT[:, :len(dev_ids)] = xT[:, dev_ids]
        in_maps.append({
            "xg": _tile_x(xgT),
            "xs": _tile_x(xT[:, e * TSH:(e + 1) * TSH]),
            "wg": _tile_w(wgp[e]),
            "wu": _tile_w(wup[e]),
            "wd": _tile_w(wdp[e]),
            "wsg": wsg_t,
            "wsu": wsu_t,
            "wsd": wsd_t,
        })

    nc = _get_nc()
    res = run_bass_kernel_spmd(nc, in_maps, core_ids=list(range(8))).results

    out = np.empty((T, H), dtype=np.float32)
    for c in range(8):
        out[c * TSH:(c + 1) * TSH] = res[c]["shs"].T
    for e in range(E):
        n = len(tok_ids[e])
        if n:
            out[tok_ids[e]] += (res[e]["yexp"][:, :n] * tok_w[e][None, :]).T
        if len(over_ids[e]):
            y = _np_expert(x[over_ids[e]], wgp[e], wup[e], wdp[e])
            out[over_ids[e]] += over_w[e][:, None] * y

    return out.reshape(1, T, H), np.float32(0.0)
